# revision 1
# baseline (speedup 1.0000x reference)
"""GRUAggregation1d Trainium2 kernel.

Computes, for xs [B=16, 512, L=8192], z_prev [B, 128, L] (all fp32):
    q  = sigmoid(Wq@xs + Uq@z + bq)        (per position l, batch b)
    r  = sigmoid(Wr@xs + Ur@z + br)
    zt = tanh(Wz@xs + Uz@(r*z) + bz)
    out = q*z + (1-q)*zt

Sharding: data-parallel over batch. 8 cores x 2 batches each; weights
replicated. Each core loops over 2 batches x 16 position-tiles of 512.
Per tile: 15 matmuls (3 gates x (4 K-chunks of W + 1 U matmul)) accumulated
in PSUM, sigmoid/tanh on ScalarE (bias fused), gate combine on VectorE.
One-iteration software pipeline: the Uz@(r*z) matmul + tanh + combine of
tile i are emitted during tile i+1 so the PE never stalls on the
r -> r*z dependency chain.

Matmul inputs are bf16 (xs and the weights are cast on the host, halving
the xs DMA; z_prev is DMA'd fp32 and cast to bf16 on ScalarE so the final
combine q*z + (1-q)*zt still sees fp32 z). PSUM accumulation is fp32.
"""

from contextlib import ExitStack

import ml_dtypes
import numpy as np

import concourse.bass as bass
import concourse.mybir as mybir
import concourse.tile as tile
from concourse import bacc
from concourse.bass_utils import run_bass_kernel_spmd

B, IN_DIM, WIDTH, L = 16, 512, 128, 8192
N_CORES = 8
B_PER = B // N_CORES          # batches per core
KC = IN_DIM // 128            # K chunks for the W matmuls
NT = 512                      # positions per tile
N_LT = L // NT                # position tiles per batch
F32 = mybir.dt.float32
BF16 = mybir.dt.bfloat16

_module_cache = {}


def _build():
    key = ("bf16", NT)
    if key in _module_cache:
        return _module_cache[key]

    nc = bacc.Bacc("TRN2", target_bir_lowering=False, debug=False,
                   num_devices=N_CORES)

    xs_d = nc.dram_tensor("xs", [B_PER, IN_DIM, L], BF16, kind="ExternalInput").ap()
    zp_d = nc.dram_tensor("zp", [B_PER, WIDTH, L], F32, kind="ExternalInput").ap()
    w_d = {}
    for g in ("q", "r", "z"):
        w_d[f"w{g}"] = nc.dram_tensor(f"w{g}", [128, KC, 128], BF16,
                                      kind="ExternalInput").ap()
        w_d[f"u{g}"] = nc.dram_tensor(f"u{g}", [128, 128], BF16,
                                      kind="ExternalInput").ap()
        w_d[f"b{g}"] = nc.dram_tensor(f"b{g}", [128, 1], F32,
                                      kind="ExternalInput").ap()
    out_d = nc.dram_tensor("out", [B_PER, WIDTH, L], F32, kind="ExternalOutput").ap()

    # [b, (k p), l] -> [b, p, k, l] so a position-tile slice is a [128, KC, NT]
    # DMA with 1KB contiguous rows
    xs_r = xs_d.rearrange("b (k p) l -> b p k l", p=128)

    with tile.TileContext(nc) as tc, ExitStack() as ctx:
        wpool = ctx.enter_context(tc.tile_pool(name="weights", bufs=1))
        io = ctx.enter_context(tc.tile_pool(name="io", bufs=3))
        acts = ctx.enter_context(tc.tile_pool(name="acts", bufs=3))
        psum = ctx.enter_context(tc.tile_pool(name="psum", bufs=2, space="PSUM"))

        # weights + biases, loaded once
        w_sb = {}
        for g in ("q", "r", "z"):
            wt = wpool.tile([128, KC, 128], BF16, tag=f"w{g}")
            nc.sync.dma_start(wt[:], w_d[f"w{g}"][:])
            ut = wpool.tile([128, 128], BF16, tag=f"u{g}")
            nc.sync.dma_start(ut[:], w_d[f"u{g}"][:])
            bt = wpool.tile([128, 1], F32, tag=f"b{g}")
            nc.sync.dma_start(bt[:], w_d[f"b{g}"][:])
            w_sb[g] = (wt, ut, bt)

        # one software-pipeline stage of carried state per tile:
        # (zt_psum, rz, q_sbuf, z_sbuf, out_slice)
        carry = None

        def finish_prev(carry):
            """Emit the trailing half of tile i (Uz matmul, tanh, combine,
            store) -- called while tile i+1's leading half is in flight."""
            zt_ps, rz, q_s, z_t, out_slice = carry
            _, uz_t, bz_t = w_sb["z"]
            nc.tensor.matmul(zt_ps[:], uz_t[:], rz[:], start=False, stop=True)
            zt_s = acts.tile([128, NT], F32, tag="zt_s")
            nc.scalar.activation(zt_s[:], zt_ps[:],
                                 mybir.ActivationFunctionType.Tanh, bias=bz_t[:])
            # out = zt + q*(z - zt)
            diff = acts.tile([128, NT], F32, tag="diff")
            nc.vector.tensor_sub(diff[:], z_t[:], zt_s[:])
            prod = acts.tile([128, NT], F32, tag="prod")
            nc.vector.tensor_mul(prod[:], q_s[:], diff[:])
            o_t = acts.tile([128, NT], F32, tag="o_t")
            nc.vector.tensor_add(o_t[:], zt_s[:], prod[:])
            nc.sync.dma_start(out_slice, o_t[:])

        for b in range(B_PER):
            for i in range(N_LT):
                l0 = i * NT
                xs_t = io.tile([128, KC, NT], BF16, tag="xs_t")
                nc.sync.dma_start(xs_t[:], xs_r[b][:, :, l0:l0 + NT])
                z_t = io.tile([128, NT], F32, tag="z_t")
                nc.sync.dma_start(z_t[:], zp_d[b][:, l0:l0 + NT])
                # bf16 copy of z for the U matmuls (ScalarE has spare cycles)
                z_bf = io.tile([128, NT], BF16, tag="z_bf")
                nc.scalar.activation(z_bf[:], z_t[:],
                                     mybir.ActivationFunctionType.Copy)

                if carry is not None:
                    finish_prev(carry)
                    carry = None

                # ---- q gate ----
                wq_t, uq_t, bq_t = w_sb["q"]
                q_ps = psum.tile([128, NT], F32, tag="q_ps")
                for k in range(KC):
                    nc.tensor.matmul(q_ps[:], wq_t[:, k, :], xs_t[:, k, :],
                                     start=(k == 0), stop=False)
                nc.tensor.matmul(q_ps[:], uq_t[:], z_bf[:], start=False, stop=True)
                q_s = acts.tile([128, NT], F32, tag="q_s")
                nc.scalar.activation(q_s[:], q_ps[:],
                                     mybir.ActivationFunctionType.Sigmoid,
                                     bias=bq_t[:])

                # ---- r gate ----
                wr_t, ur_t, br_t = w_sb["r"]
                r_ps = psum.tile([128, NT], F32, tag="r_ps")
                for k in range(KC):
                    nc.tensor.matmul(r_ps[:], wr_t[:, k, :], xs_t[:, k, :],
                                     start=(k == 0), stop=False)
                nc.tensor.matmul(r_ps[:], ur_t[:], z_bf[:], start=False, stop=True)
                r_s = acts.tile([128, NT], BF16, tag="r_s")
                nc.scalar.activation(r_s[:], r_ps[:],
                                     mybir.ActivationFunctionType.Sigmoid,
                                     bias=br_t[:])

                # ---- zt: W part only; Uz@(r*z) lands next iteration ----
                wz_t, _, _ = w_sb["z"]
                zt_ps = psum.tile([128, NT], F32, tag="zt_ps")
                for k in range(KC):
                    nc.tensor.matmul(zt_ps[:], wz_t[:, k, :], xs_t[:, k, :],
                                     start=(k == 0), stop=False)

                rz = acts.tile([128, NT], BF16, tag="rz")
                nc.vector.tensor_mul(rz[:], r_s[:], z_bf[:])
                carry = (zt_ps, rz, q_s, z_t, out_d[b][:, l0:l0 + NT])

        finish_prev(carry)

    nc.compile()
    _module_cache[key] = nc
    return nc


def _pack_w(w):
    # W [128 out, 512 in] -> [128 part=in%128, KC, 128 out]
    return np.ascontiguousarray(
        w.T.reshape(KC, 128, 128).transpose(1, 0, 2)).astype(ml_dtypes.bfloat16)


def _run(inputs, trace=False, **run_kwargs):
    xs = np.asarray(inputs["xs"], dtype=np.float32)
    zp = np.ascontiguousarray(np.asarray(inputs["z_prev"], dtype=np.float32))
    assert xs.shape == (B, IN_DIM, L) and zp.shape == (B, WIDTH, L)
    xs_bf = np.ascontiguousarray(xs.astype(ml_dtypes.bfloat16))

    packed = {}
    for g, (wn, un, wbn, ubn) in {
        "q": ("Wq_w", "Uq_w", "Wq_b", "Uq_b"),
        "r": ("Wr_w", "Ur_w", "Wr_b", "Ur_b"),
        "z": ("Wz_w", "Uz_w", "Wz_b", "Uz_b"),
    }.items():
        packed[f"w{g}"] = _pack_w(np.asarray(inputs[wn], dtype=np.float32))
        packed[f"u{g}"] = np.ascontiguousarray(
            np.asarray(inputs[un], dtype=np.float32).T.astype(ml_dtypes.bfloat16))
        packed[f"b{g}"] = np.ascontiguousarray(
            (np.asarray(inputs[wbn], dtype=np.float32)
             + np.asarray(inputs[ubn], dtype=np.float32)).reshape(128, 1))

    nc = _build()
    in_maps = []
    for c in range(N_CORES):
        m = {"xs": np.ascontiguousarray(xs_bf[c * B_PER:(c + 1) * B_PER]),
             "zp": np.ascontiguousarray(zp[c * B_PER:(c + 1) * B_PER])}
        m.update(packed)
        in_maps.append(m)

    res = run_bass_kernel_spmd(nc, in_maps, core_ids=list(range(N_CORES)),
                               trace=trace, **run_kwargs)
    out = np.concatenate([res.results[c]["out"] for c in range(N_CORES)], axis=0)
    return out, res


def kernel(**inputs):
    out, _ = _run(inputs, trace=False)
    return out



# revision 5
# speedup vs baseline: 1.0359x; 1.0359x over previous
"""GRUAggregation1d Trainium2 kernel (v2).

Computes, for xs [B=16, 512, L=8192], z_prev [B, 128, L] (all fp32):
    q  = sigmoid(Wq@xs + Uq@z + bq)        (per position l, batch b)
    r  = sigmoid(Wr@xs + Ur@z + br)
    zt = tanh(Wz@xs + Uz@(r*z) + bz)
    out = q*z + (1-q)*zt

Sharding: data-parallel over batch. 8 cores x 2 batches each; weights
replicated.

v2 design (vs v1 baseline at ~160us):
- All matmul inputs bf16 (fp8 was measured numerically out of tolerance for
  the q/zt paths), PSUM fp32, N=512 per matmul.
- DMA in 2048-position chunks (4KB contiguous rows), 3 triggers per chunk
  instead of per-512-tile triggers: the sync queue was spending ~77us just
  executing DMA_DIRECT2D triggers. Out-store triggers go on the idle GpSimd
  queue.
- z_prev DMA'd as bf16 (host cast) and the final combine runs in bf16 on
  DVE at 2x rate; out stored bf16 (host casts back to fp32). Kills the
  per-tile fp32->bf16 ACT copy and 8.4MB of DMA.
- ACT/DVE ops are 1024 wide (supertile = 2 PSUM banks) to amortize the
  fixed per-op overhead; ACT does sigmoid/tanh with fused bias reading
  fp32 PSUM, writing bf16.
- Matmul order per supertile alternates PSUM banks between consecutive
  matmuls and keeps the two half-tile matmuls of the same weight adjacent
  (weight-load reuse; LDWEIGHTS overlap confirmed on HW).
- All 9 weight tensors packed into one [128,1920] bf16 DMA + one [128,3]
  fp32 bias DMA.
- Software pipeline: Uz@(r*z) + tanh + combine + store of supertile i are
  emitted during supertile i+1 so the PE never waits on the r->r*z chain.
- PSUM budget: q bufs=1 (2 banks) + r bufs=1 (2) + zt bufs=2 (4) = 8.
"""

from contextlib import ExitStack

import ml_dtypes
import numpy as np

import concourse.bass as bass
import concourse.mybir as mybir
import concourse.tile as tile
from concourse import bacc
from concourse.bass_utils import run_bass_kernel_spmd

B, IN_DIM, WIDTH, L = 16, 512, 128, 8192
N_CORES = 8
B_PER = B // N_CORES          # batches per core
KC = IN_DIM // 128            # K chunks for the W matmuls
NT = 512                      # positions per matmul (one PSUM bank)
ST = 1024                     # supertile: ACT/DVE op width (2 banks)
CH = 2048                     # DMA chunk positions
N_CH = L // CH                # chunks per batch
F32 = mybir.dt.float32
BF16 = mybir.dt.bfloat16

_module_cache = {}


def _build():
    key = ("v2", CH, ST)
    if key in _module_cache:
        return _module_cache[key]

    nc = bacc.Bacc("TRN2", target_bir_lowering=False, debug=False,
                   num_devices=N_CORES)

    xs_d = nc.dram_tensor("xs", [B_PER, IN_DIM, L], BF16, kind="ExternalInput").ap()
    zp_d = nc.dram_tensor("zp", [B_PER, WIDTH, L], BF16, kind="ExternalInput").ap()
    wp_d = nc.dram_tensor("wp", [128, 1920], BF16, kind="ExternalInput").ap()
    bp_d = nc.dram_tensor("bp", [128, 3], F32, kind="ExternalInput").ap()
    out_d = nc.dram_tensor("out", [B_PER, WIDTH, L], BF16,
                           kind="ExternalOutput").ap()

    # [b, (k p), l] -> [b, p, k, l]: a chunk slice is a [128, KC, CH] DMA
    # with 4KB contiguous rows
    xs_r = xs_d.rearrange("b (k p) l -> b p k l", p=128)

    with tile.TileContext(nc) as tc, ExitStack() as ctx:
        wpool = ctx.enter_context(tc.tile_pool(name="weights", bufs=1))
        io = ctx.enter_context(tc.tile_pool(name="io", bufs=2))
        obuf = ctx.enter_context(tc.tile_pool(name="obuf", bufs=2))
        acts = ctx.enter_context(tc.tile_pool(name="acts", bufs=2))
        ps_qr = ctx.enter_context(tc.tile_pool(name="ps_qr", bufs=1,
                                               space="PSUM"))
        ps_z = ctx.enter_context(tc.tile_pool(name="ps_z", bufs=2,
                                              space="PSUM"))

        # ---- first chunk's inputs, then weights (both small); first xs
        # half goes first so the PE can start ASAP.
        xs_t0 = io.tile([128, KC, CH], BF16, tag="xs_t")
        nc.sync.dma_start(xs_t0[:, :, 0:ST], xs_r[0][:, :, 0:ST])
        w_sb = wpool.tile([128, 1920], BF16, tag="wp")
        nc.sync.dma_start(w_sb[:], wp_d[:])
        b_sb = wpool.tile([128, 3], F32, tag="bp")
        nc.sync.dma_start(b_sb[:], bp_d[:])
        z_t0 = io.tile([128, CH], BF16, tag="z_t")
        nc.sync.dma_start(z_t0[:], zp_d[0][:, 0:CH])
        nc.sync.dma_start(xs_t0[:, :, ST:CH], xs_r[0][:, :, ST:CH])

        # weight slices: per gate g (0=q,1=r,2=z): W chunks at
        # [:, g*512 + k*128 : +128], U at [:, 1536 + g*128 : +128]
        def wslice(g, k):
            return w_sb[:, g * 512 + k * 128: g * 512 + (k + 1) * 128]

        def uslice(g):
            return w_sb[:, 1536 + g * 128: 1536 + (g + 1) * 128]

        # one software-pipeline stage of carried state per supertile:
        # (zt_ps, rz, q_s, z_tile, s0, outbuf, flush_out)
        carry = None

        def finish_prev(c):
            """Trailing half of supertile i (Uz matmuls, tanh, combine,
            maybe chunk store) -- emitted during supertile i+1."""
            zt_ps, rz, q_s, z_t, s0, ob, flush = c
            uz = uslice(2)
            for h in range(2):
                nc.tensor.matmul(zt_ps[:, h * NT:(h + 1) * NT], uz,
                                 rz[:, h * NT:(h + 1) * NT],
                                 start=False, stop=True)
            zt_s = acts.tile([128, ST], BF16, tag="zt_s")
            nc.scalar.activation(zt_s[:], zt_ps[:],
                                 mybir.ActivationFunctionType.Tanh,
                                 bias=b_sb[:, 2:3])
            # out = zt + q*(z - zt), all bf16 on DVE (2x rate)
            diff = acts.tile([128, ST], BF16, tag="diff")
            nc.vector.tensor_sub(diff[:], z_t[:, s0:s0 + ST], zt_s[:])
            prod = acts.tile([128, ST], BF16, tag="prod")
            nc.vector.tensor_mul(prod[:], q_s[:], diff[:])
            nc.vector.tensor_add(ob[:, s0:s0 + ST], zt_s[:], prod[:])
            if flush is not None:
                b_i, l0 = flush
                nc.gpsimd.dma_start(out_d[b_i][:, l0:l0 + CH], ob[:])

        chunks = [(b_i, ci) for b_i in range(B_PER) for ci in range(N_CH)]
        for n, (b_i, ci) in enumerate(chunks):
            l0 = ci * CH
            if n == 0:
                xs_t, z_t = xs_t0, z_t0
            else:
                xs_t = io.tile([128, KC, CH], BF16, tag="xs_t")
                nc.sync.dma_start(xs_t[:], xs_r[b_i][:, :, l0:l0 + CH])
                z_t = io.tile([128, CH], BF16, tag="z_t")
                nc.sync.dma_start(z_t[:], zp_d[b_i][:, l0:l0 + CH])
            ob = obuf.tile([128, CH], BF16, tag="ob")

            for s in range(CH // ST):
                s0 = s * ST
                q_ps = ps_qr.tile([128, ST], F32, tag="q_ps")
                r_ps = ps_qr.tile([128, ST], F32, tag="r_ps")
                zt_ps = ps_z.tile([128, ST], F32, tag="zt_ps")

                if carry is not None:
                    finish_prev(carry)
                    carry = None

                # W matmuls: k-major, gates interleaved, the two half-tiles
                # of one weight adjacent -> consecutive matmuls never target
                # the same PSUM bank and each weight serves 2 matmuls.
                for k in range(KC):
                    for g, ps in ((0, q_ps), (1, r_ps), (2, zt_ps)):
                        w = wslice(g, k)
                        for h in range(2):
                            nc.tensor.matmul(
                                ps[:, h * NT:(h + 1) * NT], w,
                                xs_t[:, k, s0 + h * NT: s0 + (h + 1) * NT],
                                start=(k == 0), stop=False)
                # U matmuls for q and r (zt's U lands next supertile)
                for g, ps in ((0, q_ps), (1, r_ps)):
                    u = uslice(g)
                    for h in range(2):
                        nc.tensor.matmul(
                            ps[:, h * NT:(h + 1) * NT], u,
                            z_t[:, s0 + h * NT: s0 + (h + 1) * NT],
                            start=False, stop=True)

                q_s = acts.tile([128, ST], BF16, tag="q_s")
                nc.scalar.activation(q_s[:], q_ps[:],
                                     mybir.ActivationFunctionType.Sigmoid,
                                     bias=b_sb[:, 0:1])
                r_s = acts.tile([128, ST], BF16, tag="r_s")
                nc.scalar.activation(r_s[:], r_ps[:],
                                     mybir.ActivationFunctionType.Sigmoid,
                                     bias=b_sb[:, 1:2])
                rz = acts.tile([128, ST], BF16, tag="rz")
                nc.vector.tensor_mul(rz[:], r_s[:], z_t[:, s0:s0 + ST])

                flush = (b_i, l0) if s == CH // ST - 1 else None
                carry = (zt_ps, rz, q_s, z_t, s0, ob, flush)

        finish_prev(carry)

    nc.compile()
    _module_cache[key] = nc
    return nc


def _pack_weights(inputs):
    # wp [128, 1920] bf16: per partition p:
    #   [g=q,r,z][k=0..3]: wp[p, g*512+k*128+o] = Wg_w[o, k*128+p]
    #   [g]: wp[p, 1536+g*128+o] = Ug_w[o, p]
    wp = np.empty((128, 1920), np.float32)
    bp = np.empty((128, 3), np.float32)
    for g, (wn, un, wbn, ubn) in enumerate((
        ("Wq_w", "Uq_w", "Wq_b", "Uq_b"),
        ("Wr_w", "Ur_w", "Wr_b", "Ur_b"),
        ("Wz_w", "Uz_w", "Wz_b", "Uz_b"),
    )):
        w = np.asarray(inputs[wn], np.float32)       # [128 out, 512 in]
        # [o, k*128+p] -> [p, k, o]
        wp[:, g * 512:(g + 1) * 512] = (
            w.reshape(128, KC, 128).transpose(2, 1, 0).reshape(128, 512))
        wp[:, 1536 + g * 128: 1536 + (g + 1) * 128] = (
            np.asarray(inputs[un], np.float32).T)
        bp[:, g] = (np.asarray(inputs[wbn], np.float32)
                    + np.asarray(inputs[ubn], np.float32))
    return (np.ascontiguousarray(wp.astype(ml_dtypes.bfloat16)),
            np.ascontiguousarray(bp))


def _run(inputs, trace=False, **run_kwargs):
    xs = np.asarray(inputs["xs"], dtype=np.float32)
    zp = np.asarray(inputs["z_prev"], dtype=np.float32)
    assert xs.shape == (B, IN_DIM, L) and zp.shape == (B, WIDTH, L)
    xs_bf = np.ascontiguousarray(xs.astype(ml_dtypes.bfloat16))
    zp_bf = np.ascontiguousarray(zp.astype(ml_dtypes.bfloat16))
    wp, bp = _pack_weights(inputs)

    nc = _build()
    in_maps = []
    for c in range(N_CORES):
        m = {"xs": np.ascontiguousarray(xs_bf[c * B_PER:(c + 1) * B_PER]),
             "zp": np.ascontiguousarray(zp_bf[c * B_PER:(c + 1) * B_PER]),
             "wp": wp, "bp": bp}
        in_maps.append(m)

    res = run_bass_kernel_spmd(nc, in_maps, core_ids=list(range(N_CORES)),
                               trace=trace, **run_kwargs)
    out = np.concatenate(
        [np.asarray(res.results[c]["out"], dtype=np.float32)
         for c in range(N_CORES)], axis=0)
    return out, res


def kernel(**inputs):
    out, _ = _run(inputs, trace=False)
    return out


# revision 12
# speedup vs baseline: 1.1556x; 1.1155x over previous
"""GRUAggregation1d Trainium2 kernel (v2).

Computes, for xs [B=16, 512, L=8192], z_prev [B, 128, L] (all fp32):
    q  = sigmoid(Wq@xs + Uq@z + bq)        (per position l, batch b)
    r  = sigmoid(Wr@xs + Ur@z + br)
    zt = tanh(Wz@xs + Uz@(r*z) + bz)
    out = q*z + (1-q)*zt

Sharding: data-parallel over batch. 8 cores x 2 batches each; weights
replicated.

v2 design (vs v1 baseline at ~160us):
- All matmul inputs bf16 (fp8 was measured numerically out of tolerance for
  the q/zt paths), PSUM fp32, N=512 per matmul.
- DMA in 2048-position chunks (4KB contiguous rows), 3 triggers per chunk
  instead of per-512-tile triggers: the sync queue was spending ~77us just
  executing DMA_DIRECT2D triggers. Out-store triggers go on the idle GpSimd
  queue.
- z_prev DMA'd as bf16 (host cast) and the final combine runs in bf16 on
  DVE at 2x rate; out stored bf16 (host casts back to fp32). Kills the
  per-tile fp32->bf16 ACT copy and 8.4MB of DMA.
- ACT/DVE ops are 1024 wide (supertile = 2 PSUM banks) to amortize the
  fixed per-op overhead; ACT does sigmoid/tanh with fused bias reading
  fp32 PSUM, writing bf16.
- Matmul order per supertile alternates PSUM banks between consecutive
  matmuls and keeps the two half-tile matmuls of the same weight adjacent
  (weight-load reuse; LDWEIGHTS overlap confirmed on HW).
- All 9 weight tensors packed into one [128,1920] bf16 DMA + one [128,3]
  fp32 bias DMA.
- Software pipeline: Uz@(r*z) + tanh + combine + store of supertile i are
  emitted during supertile i+1 so the PE never waits on the r->r*z chain.
- PSUM budget: q bufs=1 (2 banks) + r bufs=1 (2) + zt bufs=2 (4) = 8.
"""

from contextlib import ExitStack

import ml_dtypes
import numpy as np

import concourse.bass as bass
import concourse.mybir as mybir
import concourse.tile as tile
from concourse import bacc
from concourse.bass_utils import run_bass_kernel_spmd

B, IN_DIM, WIDTH, L = 16, 512, 128, 8192
N_CORES = 8
B_PER = B // N_CORES          # batches per core
KC = IN_DIM // 128            # K chunks for the W matmuls
NT = 512                      # positions per matmul (one PSUM bank)
ST = 1024                     # supertile: ACT/DVE op width (2 banks)
CH = 2048                     # DMA chunk positions
N_CH = L // CH                # chunks per batch
F32 = mybir.dt.float32
BF16 = mybir.dt.bfloat16
FP8 = mybir.dt.float8e4
WRS = 64.0                    # r-gate fp8 weight pre-scale

_module_cache = {}


def _build():
    key = ("v2", CH, ST)
    if key in _module_cache:
        return _module_cache[key]

    nc = bacc.Bacc("TRN2", target_bir_lowering=False, debug=False,
                   num_devices=N_CORES)

    xs_d = nc.dram_tensor("xs", [B_PER, IN_DIM, L], BF16, kind="ExternalInput").ap()
    x8_d = nc.dram_tensor("x8", [B_PER, IN_DIM, L], FP8, kind="ExternalInput").ap()
    zp_d = nc.dram_tensor("zp", [B_PER, WIDTH, L], BF16, kind="ExternalInput").ap()
    wp_d = nc.dram_tensor("wp", [128, 1920], BF16, kind="ExternalInput").ap()
    w8_d = nc.dram_tensor("w8", [128, 2, 2, 128], FP8, kind="ExternalInput").ap()
    bp_d = nc.dram_tensor("bp", [128, 3], F32, kind="ExternalInput").ap()
    out_d = nc.dram_tensor("out", [B_PER, WIDTH, L], BF16,
                           kind="ExternalOutput").ap()

    # [b, (k p), l] -> [b, p, k, l]: a chunk slice is a [128, KC, CH] DMA
    # with 4KB contiguous rows
    xs_r = xs_d.rearrange("b (k p) l -> b p k l", p=128)
    x8_r = x8_d.rearrange("b (k p) l -> b p k l", p=128)

    with tile.TileContext(nc) as tc, ExitStack() as ctx:
        wpool = ctx.enter_context(tc.tile_pool(name="weights", bufs=1))
        io = ctx.enter_context(tc.tile_pool(name="io", bufs=2))
        obuf = ctx.enter_context(tc.tile_pool(name="obuf", bufs=2))
        acts = ctx.enter_context(tc.tile_pool(name="acts", bufs=2))
        ps_qr = ctx.enter_context(tc.tile_pool(name="ps_qr", bufs=1,
                                               space="PSUM"))
        ps_z = ctx.enter_context(tc.tile_pool(name="ps_z", bufs=2,
                                              space="PSUM"))

        # ---- weights first (small, every matmul needs them), then the
        # first supertile's inputs so the PE can start ASAP.
        w_sb = wpool.tile([128, 1920], BF16, tag="wp")
        nc.sync.dma_start(w_sb[:], wp_d[:])
        w8_sb = wpool.tile([128, 2, 2, 128], FP8, tag="w8")
        nc.sync.dma_start(w8_sb[:], w8_d[:])
        b_sb = wpool.tile([128, 3], F32, tag="bp")
        nc.sync.dma_start(b_sb[:], bp_d[:])
        xs_t0 = io.tile([128, KC, CH], BF16, tag="xs_t")
        nc.sync.dma_start(xs_t0[:, :, 0:ST], xs_r[0][:, :, 0:ST])
        x8_t0 = io.tile([128, KC, CH], FP8, tag="x8_t")
        nc.sync.dma_start(x8_t0[:, :, 0:ST], x8_r[0][:, :, 0:ST])
        z_t0 = io.tile([128, CH], BF16, tag="z_t")
        nc.sync.dma_start(z_t0[:], zp_d[0][:, 0:CH])
        nc.sync.dma_start(xs_t0[:, :, ST:CH], xs_r[0][:, :, ST:CH])
        nc.sync.dma_start(x8_t0[:, :, ST:CH], x8_r[0][:, :, ST:CH])

        # weight slices: per gate g (0=q,1=r,2=z): W chunks at
        # [:, g*512 + k*128 : +128], U at [:, 1536 + g*128 : +128]
        def wslice(g, k):
            return w_sb[:, g * 512 + k * 128: g * 512 + (k + 1) * 128]

        def uslice(g):
            return w_sb[:, 1536 + g * 128: 1536 + (g + 1) * 128]

        # one software-pipeline stage of carried state per supertile:
        # (zt_ps, rz, q_s, z_tile, s0, outbuf, flush_out)
        carry = None

        def finish_prev(c):
            """Trailing half of supertile i (Uz matmuls, tanh, combine,
            maybe chunk store) -- emitted during supertile i+1."""
            zt_ps, rz, q_s, z_t, s0, ob, flush = c
            uz = uslice(2)
            for h in range(2):
                nc.tensor.matmul(zt_ps[:, h * NT:(h + 1) * NT], uz,
                                 rz[:, h * NT:(h + 1) * NT],
                                 start=False, stop=True)
            zt_s = acts.tile([128, ST], BF16, tag="zt_s")
            nc.scalar.activation(zt_s[:], zt_ps[:],
                                 mybir.ActivationFunctionType.Tanh,
                                 bias=b_sb[:, 2:3])
            # out = zt + q*(z - zt), all bf16 on DVE (2x rate)
            diff = acts.tile([128, ST], BF16, tag="diff")
            nc.vector.tensor_sub(diff[:], z_t[:, s0:s0 + ST], zt_s[:])
            prod = acts.tile([128, ST], BF16, tag="prod")
            nc.vector.tensor_mul(prod[:], q_s[:], diff[:])
            nc.vector.tensor_add(ob[:, s0:s0 + ST], zt_s[:], prod[:])
            if flush is not None:
                b_i, l0 = flush
                nc.gpsimd.dma_start(out_d[b_i][:, l0:l0 + CH], ob[:])

        chunks = [(b_i, ci) for b_i in range(B_PER) for ci in range(N_CH)]
        for n, (b_i, ci) in enumerate(chunks):
            l0 = ci * CH
            if n == 0:
                xs_t, x8_t, z_t = xs_t0, x8_t0, z_t0
            else:
                xs_t = io.tile([128, KC, CH], BF16, tag="xs_t")
                nc.sync.dma_start(xs_t[:], xs_r[b_i][:, :, l0:l0 + CH])
                x8_t = io.tile([128, KC, CH], FP8, tag="x8_t")
                nc.sync.dma_start(x8_t[:], x8_r[b_i][:, :, l0:l0 + CH])
                z_t = io.tile([128, CH], BF16, tag="z_t")
                nc.sync.dma_start(z_t[:], zp_d[b_i][:, l0:l0 + CH])
            ob = obuf.tile([128, CH], BF16, tag="ob")

            for s in range(CH // ST):
                s0 = s * ST
                q_ps = ps_qr.tile([128, ST], F32, tag="q_ps")
                r_ps = ps_qr.tile([128, ST], F32, tag="r_ps")
                zt_ps = ps_z.tile([128, ST], F32, tag="zt_ps")

                if carry is not None:
                    finish_prev(carry)
                    carry = None

                # W matmuls: k-major, gates interleaved, the two half-tiles
                # of one weight adjacent -> consecutive matmuls never target
                # the same PSUM bank and each weight serves 2 matmuls.
                # q/zt are bf16; r runs fp8 DoubleRow (K=256 per pass, x64
                # pre-scaled weights, compensated in the sigmoid's scale).
                for k in range(KC):
                    w = wslice(0, k)
                    for h in range(2):
                        nc.tensor.matmul(
                            q_ps[:, h * NT:(h + 1) * NT], w,
                            xs_t[:, k, s0 + h * NT: s0 + (h + 1) * NT],
                            start=(k == 0), stop=False)
                    if k % 2 == 0:
                        k2 = k // 2
                        for h in range(2):
                            nc.tensor.matmul(
                                r_ps[:, h * NT:(h + 1) * NT],
                                w8_sb[:, k2],
                                x8_t[:, 2 * k2:2 * k2 + 2,
                                     s0 + h * NT: s0 + (h + 1) * NT],
                                start=(k2 == 0), stop=False,
                                perf_mode=mybir.MatmulPerfMode.DoubleRow)
                    w = wslice(2, k)
                    for h in range(2):
                        nc.tensor.matmul(
                            zt_ps[:, h * NT:(h + 1) * NT], w,
                            xs_t[:, k, s0 + h * NT: s0 + (h + 1) * NT],
                            start=(k == 0), stop=False)
                # U matmuls for q and r (zt's U lands next supertile)
                for g, ps in ((0, q_ps), (1, r_ps)):
                    u = uslice(g)
                    for h in range(2):
                        nc.tensor.matmul(
                            ps[:, h * NT:(h + 1) * NT], u,
                            z_t[:, s0 + h * NT: s0 + (h + 1) * NT],
                            start=False, stop=True)

                q_s = acts.tile([128, ST], BF16, tag="q_s")
                nc.scalar.activation(q_s[:], q_ps[:],
                                     mybir.ActivationFunctionType.Sigmoid,
                                     bias=b_sb[:, 0:1])
                r_s = acts.tile([128, ST], BF16, tag="r_s")
                nc.scalar.activation(r_s[:], r_ps[:],
                                     mybir.ActivationFunctionType.Sigmoid,
                                     bias=b_sb[:, 1:2], scale=1.0 / WRS)
                rz = acts.tile([128, ST], BF16, tag="rz")
                nc.vector.tensor_mul(rz[:], r_s[:], z_t[:, s0:s0 + ST])

                flush = (b_i, l0) if s == CH // ST - 1 else None
                carry = (zt_ps, rz, q_s, z_t, s0, ob, flush)

        finish_prev(carry)

    nc.compile()
    _module_cache[key] = nc
    return nc


def _pack_weights(inputs):
    # wp [128, 1920] bf16: per partition p:
    #   [g=q,r,z][k=0..3]: wp[p, g*512+k*128+o] = Wg_w[o, k*128+p]
    #   [g]: wp[p, 1536+g*128+o] = Ug_w[o, p]
    wp = np.empty((128, 1920), np.float32)
    bp = np.empty((128, 3), np.float32)
    for g, (wn, un, wbn, ubn) in enumerate((
        ("Wq_w", "Uq_w", "Wq_b", "Uq_b"),
        ("Wr_w", "Ur_w", "Wr_b", "Ur_b"),
        ("Wz_w", "Uz_w", "Wz_b", "Uz_b"),
    )):
        w = np.asarray(inputs[wn], np.float32)       # [128 out, 512 in]
        # [o, k*128+p] -> [p, k, o]
        wp[:, g * 512:(g + 1) * 512] = (
            w.reshape(128, KC, 128).transpose(2, 1, 0).reshape(128, 512))
        # Ur is pre-scaled by WRS so its products match the scaled fp8
        # Wr products in PSUM (sigmoid applies scale=1/WRS).
        us = WRS if g == 1 else 1.0
        wp[:, 1536 + g * 128: 1536 + (g + 1) * 128] = (
            us * np.asarray(inputs[un], np.float32).T)
        bp[:, g] = (np.asarray(inputs[wbn], np.float32)
                    + np.asarray(inputs[ubn], np.float32))
    # w8 [128, k2, j, o] fp8: WRS * Wr_w[o, (2*k2+j)*128 + p]
    wr = np.asarray(inputs["Wr_w"], np.float32)      # [128, 512]
    w8 = (WRS * wr.reshape(128, 2, 2, 128).transpose(3, 1, 2, 0))
    return (np.ascontiguousarray(wp.astype(ml_dtypes.bfloat16)),
            np.ascontiguousarray(w8.astype(ml_dtypes.float8_e4m3)),
            np.ascontiguousarray(bp))


def _run(inputs, trace=False, **run_kwargs):
    xs = np.asarray(inputs["xs"], dtype=np.float32)
    zp = np.asarray(inputs["z_prev"], dtype=np.float32)
    assert xs.shape == (B, IN_DIM, L) and zp.shape == (B, WIDTH, L)
    xs_bf = np.ascontiguousarray(xs.astype(ml_dtypes.bfloat16))
    xs_f8 = np.ascontiguousarray(xs.astype(ml_dtypes.float8_e4m3))
    zp_bf = np.ascontiguousarray(zp.astype(ml_dtypes.bfloat16))
    wp, w8, bp = _pack_weights(inputs)

    nc = _build()
    in_maps = []
    for c in range(N_CORES):
        m = {"xs": np.ascontiguousarray(xs_bf[c * B_PER:(c + 1) * B_PER]),
             "x8": np.ascontiguousarray(xs_f8[c * B_PER:(c + 1) * B_PER]),
             "zp": np.ascontiguousarray(zp_bf[c * B_PER:(c + 1) * B_PER]),
             "wp": wp, "w8": w8, "bp": bp}
        in_maps.append(m)

    res = run_bass_kernel_spmd(nc, in_maps, core_ids=list(range(N_CORES)),
                               trace=trace, **run_kwargs)
    out = np.concatenate(
        [np.asarray(res.results[c]["out"], dtype=np.float32)
         for c in range(N_CORES)], axis=0)
    return out, res


def kernel(**inputs):
    out, _ = _run(inputs, trace=False)
    return out


# revision 18
# speedup vs baseline: 1.2004x; 1.0388x over previous
"""GRUAggregation1d Trainium2 kernel (v2).

Computes, for xs [B=16, 512, L=8192], z_prev [B, 128, L] (all fp32):
    q  = sigmoid(Wq@xs + Uq@z + bq)        (per position l, batch b)
    r  = sigmoid(Wr@xs + Ur@z + br)
    zt = tanh(Wz@xs + Uz@(r*z) + bz)
    out = q*z + (1-q)*zt

Sharding: data-parallel over batch. 8 cores x 2 batches each; weights
replicated.

v2 design (vs v1 baseline at ~160us):
- All matmul inputs bf16 (fp8 was measured numerically out of tolerance for
  the q/zt paths), PSUM fp32, N=512 per matmul.
- DMA in 2048-position chunks (4KB contiguous rows), 3 triggers per chunk
  instead of per-512-tile triggers: the sync queue was spending ~77us just
  executing DMA_DIRECT2D triggers. Out-store triggers go on the idle GpSimd
  queue.
- z_prev DMA'd as bf16 (host cast) and the final combine runs in bf16 on
  DVE at 2x rate; out stored bf16 (host casts back to fp32). Kills the
  per-tile fp32->bf16 ACT copy and 8.4MB of DMA.
- ACT/DVE ops are 1024 wide (supertile = 2 PSUM banks) to amortize the
  fixed per-op overhead; ACT does sigmoid/tanh with fused bias reading
  fp32 PSUM, writing bf16.
- Matmul order per supertile alternates PSUM banks between consecutive
  matmuls and keeps the two half-tile matmuls of the same weight adjacent
  (weight-load reuse; LDWEIGHTS overlap confirmed on HW).
- All 9 weight tensors packed into one [128,1920] bf16 DMA + one [128,3]
  fp32 bias DMA.
- Software pipeline: Uz@(r*z) + tanh + combine + store of supertile i are
  emitted during supertile i+1 so the PE never waits on the r->r*z chain.
- PSUM budget: q bufs=1 (2 banks) + r bufs=1 (2) + zt bufs=2 (4) = 8.
"""

from contextlib import ExitStack

import ml_dtypes
import numpy as np

import concourse.bass as bass
import concourse.mybir as mybir
import concourse.tile as tile
from concourse import bacc
from concourse.bass_utils import run_bass_kernel_spmd

B, IN_DIM, WIDTH, L = 16, 512, 128, 8192
N_CORES = 8
B_PER = B // N_CORES          # batches per core
KC = IN_DIM // 128            # K chunks for the W matmuls
NT = 512                      # positions per matmul (one PSUM bank)
ST = 1024                     # supertile: ACT/DVE op width (2 banks)
CH = 2048                     # DMA chunk positions
N_CH = L // CH                # chunks per batch
F32 = mybir.dt.float32
BF16 = mybir.dt.bfloat16
FP8 = mybir.dt.float8e4
WRS = 64.0                    # r-gate fp8 weight pre-scale

_module_cache = {}


def _build():
    key = ("v2", CH, ST)
    if key in _module_cache:
        return _module_cache[key]

    nc = bacc.Bacc("TRN2", target_bir_lowering=False, debug=False,
                   num_devices=N_CORES)

    xs_d = nc.dram_tensor("xs", [B_PER, IN_DIM, L], BF16, kind="ExternalInput").ap()
    x8_d = nc.dram_tensor("x8", [B_PER, IN_DIM, L], FP8, kind="ExternalInput").ap()
    zp_d = nc.dram_tensor("zp", [B_PER, WIDTH, L], BF16, kind="ExternalInput").ap()
    wp_d = nc.dram_tensor("wp", [128, 1920], BF16, kind="ExternalInput").ap()
    w8_d = nc.dram_tensor("w8", [128, 2, 2, 128], FP8, kind="ExternalInput").ap()
    bp_d = nc.dram_tensor("bp", [128, 3], F32, kind="ExternalInput").ap()
    out_d = nc.dram_tensor("out", [B_PER, WIDTH, L], BF16,
                           kind="ExternalOutput").ap()

    # [b, (k p), l] -> [b, p, k, l]: a chunk slice is a [128, KC, CH] DMA
    # with 4KB contiguous rows
    xs_r = xs_d.rearrange("b (k p) l -> b p k l", p=128)
    x8_r = x8_d.rearrange("b (k p) l -> b p k l", p=128)

    with tile.TileContext(nc) as tc, ExitStack() as ctx:
        wpool = ctx.enter_context(tc.tile_pool(name="weights", bufs=1))
        io = ctx.enter_context(tc.tile_pool(name="io", bufs=3))
        obuf = ctx.enter_context(tc.tile_pool(name="obuf", bufs=2))
        acts = ctx.enter_context(tc.tile_pool(name="acts", bufs=2))
        ps_qr = ctx.enter_context(tc.tile_pool(name="ps_qr", bufs=1,
                                               space="PSUM"))
        ps_z = ctx.enter_context(tc.tile_pool(name="ps_z", bufs=2,
                                              space="PSUM"))

        # ---- weights first (small, every matmul needs them), spread over
        # three queues so the triggers don't serialize.
        w_sb = wpool.tile([128, 1920], BF16, tag="wp")
        nc.sync.dma_start(w_sb[:], wp_d[:])
        w8_sb = wpool.tile([128, 2, 2, 128], FP8, tag="w8")
        nc.gpsimd.dma_start(w8_sb[:], w8_d[:])
        b_sb = wpool.tile([128, 3], F32, tag="bp")
        nc.scalar.dma_start(b_sb[:], bp_d[:])

        # weight slices: per gate g (0=q,1=r,2=z): W chunks at
        # [:, g*512 + k*128 : +128], U at [:, 1536 + g*128 : +128]
        def wslice(g, k):
            return w_sb[:, g * 512 + k * 128: g * 512 + (k + 1) * 128]

        def uslice(g):
            return w_sb[:, 1536 + g * 128: 1536 + (g + 1) * 128]

        # one software-pipeline stage of carried state per supertile:
        # (zt_ps, rz, q_s, z_tile, s0, outbuf, flush_out)
        carry = None

        def finish_prev(c):
            """Trailing half of supertile i (Uz matmuls, tanh, combine,
            maybe chunk store) -- emitted during supertile i+1."""
            zt_ps, rz, q_s, z_t, s0, ob, flush = c
            uz = uslice(2)
            for h in range(2):
                nc.tensor.matmul(zt_ps[:, h * NT:(h + 1) * NT], uz,
                                 rz[:, h * NT:(h + 1) * NT],
                                 start=False, stop=True)
            zt_s = acts.tile([128, ST], BF16, tag="zt_s")
            nc.scalar.activation(zt_s[:], zt_ps[:],
                                 mybir.ActivationFunctionType.Tanh,
                                 bias=b_sb[:, 2:3])
            # out = zt + q*(z - zt), all bf16 on DVE (2x rate)
            diff = acts.tile([128, ST], BF16, tag="diff")
            nc.vector.tensor_sub(diff[:], z_t[:, s0:s0 + ST], zt_s[:])
            prod = acts.tile([128, ST], BF16, tag="prod")
            nc.vector.tensor_mul(prod[:], q_s[:], diff[:])
            nc.vector.tensor_add(ob[:, s0:s0 + ST], zt_s[:], prod[:])
            if flush is not None:
                fb, fl, fcs = flush
                nc.gpsimd.dma_start(out_d[fb][:, fl:fl + fcs], ob[:])

        # First two chunks are supertile-sized so the PE can start as soon
        # as ~1MB has landed; the rest use the full DMA chunk.
        chunks = [(0, 0, ST), (0, ST, ST), (0, 2 * ST, CH)]
        l = 2 * ST + CH
        while l < L:
            chunks.append((0, l, CH))
            l += CH
        chunks += [(1, l0, CH) for l0 in range(0, L, CH)]
        assert sum(c[2] for c in chunks) == B_PER * L

        for n, (b_i, l0, cs) in enumerate(chunks):
            xs_t = io.tile([128, KC, cs], BF16, tag="xs_t")
            nc.sync.dma_start(xs_t[:], xs_r[b_i][:, :, l0:l0 + cs])
            x8_t = io.tile([128, KC, cs], FP8, tag="x8_t")
            nc.gpsimd.dma_start(x8_t[:], x8_r[b_i][:, :, l0:l0 + cs])
            z_t = io.tile([128, cs], BF16, tag="z_t")
            nc.scalar.dma_start(z_t[:], zp_d[b_i][:, l0:l0 + cs])
            ob = obuf.tile([128, cs], BF16, tag="ob")

            for s in range(cs // ST):
                s0 = s * ST
                q_ps = ps_qr.tile([128, ST], F32, tag="q_ps")
                r_ps = ps_qr.tile([128, ST], F32, tag="r_ps")
                zt_ps = ps_z.tile([128, ST], F32, tag="zt_ps")

                if carry is not None:
                    finish_prev(carry)
                    carry = None

                # W matmuls: k-major, gates interleaved, the two half-tiles
                # of one weight adjacent -> consecutive matmuls never target
                # the same PSUM bank and each weight serves 2 matmuls.
                # q/zt are bf16; r runs fp8 DoubleRow (K=256 per pass, x64
                # pre-scaled weights, compensated in the sigmoid's scale).
                for k in range(KC):
                    w = wslice(0, k)
                    for h in range(2):
                        nc.tensor.matmul(
                            q_ps[:, h * NT:(h + 1) * NT], w,
                            xs_t[:, k, s0 + h * NT: s0 + (h + 1) * NT],
                            start=(k == 0), stop=False)
                    if k % 2 == 0:
                        k2 = k // 2
                        for h in range(2):
                            nc.tensor.matmul(
                                r_ps[:, h * NT:(h + 1) * NT],
                                w8_sb[:, k2],
                                x8_t[:, 2 * k2:2 * k2 + 2,
                                     s0 + h * NT: s0 + (h + 1) * NT],
                                start=(k2 == 0), stop=False,
                                perf_mode=mybir.MatmulPerfMode.DoubleRow)
                    w = wslice(2, k)
                    for h in range(2):
                        nc.tensor.matmul(
                            zt_ps[:, h * NT:(h + 1) * NT], w,
                            xs_t[:, k, s0 + h * NT: s0 + (h + 1) * NT],
                            start=(k == 0), stop=False)
                # U matmuls for q and r (zt's U lands next supertile)
                for g, ps in ((0, q_ps), (1, r_ps)):
                    u = uslice(g)
                    for h in range(2):
                        nc.tensor.matmul(
                            ps[:, h * NT:(h + 1) * NT], u,
                            z_t[:, s0 + h * NT: s0 + (h + 1) * NT],
                            start=False, stop=True)

                q_s = acts.tile([128, ST], BF16, tag="q_s")
                nc.scalar.activation(q_s[:], q_ps[:],
                                     mybir.ActivationFunctionType.Sigmoid,
                                     bias=b_sb[:, 0:1])
                r_s = acts.tile([128, ST], BF16, tag="r_s")
                nc.scalar.activation(r_s[:], r_ps[:],
                                     mybir.ActivationFunctionType.Sigmoid,
                                     bias=b_sb[:, 1:2], scale=1.0 / WRS)
                rz = acts.tile([128, ST], BF16, tag="rz")
                nc.vector.tensor_mul(rz[:], r_s[:], z_t[:, s0:s0 + ST])

                flush = (b_i, l0, cs) if s == cs // ST - 1 else None
                carry = (zt_ps, rz, q_s, z_t, s0, ob, flush)

        finish_prev(carry)

    nc.compile()
    _module_cache[key] = nc
    return nc


def _pack_weights(inputs):
    # wp [128, 1920] bf16: per partition p:
    #   [g=q,r,z][k=0..3]: wp[p, g*512+k*128+o] = Wg_w[o, k*128+p]
    #   [g]: wp[p, 1536+g*128+o] = Ug_w[o, p]
    wp = np.empty((128, 1920), np.float32)
    bp = np.empty((128, 3), np.float32)
    for g, (wn, un, wbn, ubn) in enumerate((
        ("Wq_w", "Uq_w", "Wq_b", "Uq_b"),
        ("Wr_w", "Ur_w", "Wr_b", "Ur_b"),
        ("Wz_w", "Uz_w", "Wz_b", "Uz_b"),
    )):
        w = np.asarray(inputs[wn], np.float32)       # [128 out, 512 in]
        # [o, k*128+p] -> [p, k, o]
        wp[:, g * 512:(g + 1) * 512] = (
            w.reshape(128, KC, 128).transpose(2, 1, 0).reshape(128, 512))
        # Ur is pre-scaled by WRS so its products match the scaled fp8
        # Wr products in PSUM (sigmoid applies scale=1/WRS).
        us = WRS if g == 1 else 1.0
        wp[:, 1536 + g * 128: 1536 + (g + 1) * 128] = (
            us * np.asarray(inputs[un], np.float32).T)
        bp[:, g] = (np.asarray(inputs[wbn], np.float32)
                    + np.asarray(inputs[ubn], np.float32))
    # w8 [128, k2, j, o] fp8: WRS * Wr_w[o, (2*k2+j)*128 + p]
    wr = np.asarray(inputs["Wr_w"], np.float32)      # [128, 512]
    w8 = (WRS * wr.reshape(128, 2, 2, 128).transpose(3, 1, 2, 0))
    return (np.ascontiguousarray(wp.astype(ml_dtypes.bfloat16)),
            np.ascontiguousarray(w8.astype(ml_dtypes.float8_e4m3)),
            np.ascontiguousarray(bp))


def _run(inputs, trace=False, **run_kwargs):
    xs = np.asarray(inputs["xs"], dtype=np.float32)
    zp = np.asarray(inputs["z_prev"], dtype=np.float32)
    assert xs.shape == (B, IN_DIM, L) and zp.shape == (B, WIDTH, L)
    xs_bf = np.ascontiguousarray(xs.astype(ml_dtypes.bfloat16))
    xs_f8 = np.ascontiguousarray(xs.astype(ml_dtypes.float8_e4m3))
    zp_bf = np.ascontiguousarray(zp.astype(ml_dtypes.bfloat16))
    wp, w8, bp = _pack_weights(inputs)

    nc = _build()
    in_maps = []
    for c in range(N_CORES):
        m = {"xs": np.ascontiguousarray(xs_bf[c * B_PER:(c + 1) * B_PER]),
             "x8": np.ascontiguousarray(xs_f8[c * B_PER:(c + 1) * B_PER]),
             "zp": np.ascontiguousarray(zp_bf[c * B_PER:(c + 1) * B_PER]),
             "wp": wp, "w8": w8, "bp": bp}
        in_maps.append(m)

    res = run_bass_kernel_spmd(nc, in_maps, core_ids=list(range(N_CORES)),
                               trace=trace, **run_kwargs)
    out = np.concatenate(
        [np.asarray(res.results[c]["out"], dtype=np.float32)
         for c in range(N_CORES)], axis=0)
    return out, res


def kernel(**inputs):
    out, _ = _run(inputs, trace=False)
    return out


# revision 20
# speedup vs baseline: 1.3392x; 1.1156x over previous
"""GRUAggregation1d Trainium2 kernel (v6).

Computes, for xs [B=16, 512, L=8192], z_prev [B, 128, L] (all fp32):
    q  = sigmoid(Wq@xs + Uq@z + bq)        (per position l, batch b)
    r  = sigmoid(Wr@xs + Ur@z + br)
    zt = tanh(Wz@xs + Uz@(r*z) + bz)
    out = q*z + (1-q)*zt

Sharding: data-parallel over batch. 8 cores x 2 batches each; weights
replicated.

Design (baseline ~160us -> this):
- q/zt matmuls bf16 (fp8 measured out of tolerance on those paths); the
  r gate runs fp8 DoubleRow (K=256 per pass) with x64 pre-scaled weights,
  compensated via the sigmoid's scale; Ur is pre-scaled x64 in bf16 so
  its products land on the same PSUM scale. PSUM fp32, N=512 per matmul.
- Work unit: 1024-position supertile == DMA chunk (4KB xs rows, 2KB fp8
  rows). 5-deep input prefetch so the DMA stream never starves the PE
  after the initial fill. xs/z triggers on the sync queue, x8/out on
  gpsimd, scalar reserved for ACT (DMA triggers cost ~0.7us each).
- Per supertile the r gate is computed FIRST so r*z (DVE, bf16) is ready
  ~1.3us before the Uz matmuls at the stream tail -- no cross-supertile
  software pipeline needed, and the PE never waits on the r->r*z chain.
- Consecutive matmuls always target different PSUM banks; the two
  half-tile matmuls per weight are adjacent (stationary-weight reuse;
  LDWEIGHTS overlaps matmuls in HW).
- ACT ops are 1024 wide with fused bias (PSUM fp32 in, bf16 out); the
  combine runs in bf16 on DVE (2x rate); z_prev and out are bf16 in HBM
  (host casts), halving that traffic.
- All bf16 weights in one [128,1920] DMA; fp8 r-weights + biases in two
  tiny DMAs.
"""

from contextlib import ExitStack

import ml_dtypes
import numpy as np

import concourse.bass as bass
import concourse.mybir as mybir
import concourse.tile as tile
from concourse import bacc
from concourse.bass_utils import run_bass_kernel_spmd

B, IN_DIM, WIDTH, L = 16, 512, 128, 8192
N_CORES = 8
B_PER = B // N_CORES          # batches per core
KC = IN_DIM // 128            # K chunks for the W matmuls
NT = 512                      # positions per matmul (one PSUM bank)
ST = 1024                     # supertile / DMA chunk positions
F32 = mybir.dt.float32
BF16 = mybir.dt.bfloat16
FP8 = mybir.dt.float8e4
WRS = 64.0                    # r-gate fp8 weight pre-scale

_module_cache = {}


def _build():
    key = ("v6", ST)
    if key in _module_cache:
        return _module_cache[key]

    nc = bacc.Bacc("TRN2", target_bir_lowering=False, debug=False,
                   num_devices=N_CORES)

    xs_d = nc.dram_tensor("xs", [B_PER, IN_DIM, L], BF16, kind="ExternalInput").ap()
    x8_d = nc.dram_tensor("x8", [B_PER, IN_DIM, L], FP8, kind="ExternalInput").ap()
    zp_d = nc.dram_tensor("zp", [B_PER, WIDTH, L], BF16, kind="ExternalInput").ap()
    wp_d = nc.dram_tensor("wp", [128, 1920], BF16, kind="ExternalInput").ap()
    w8_d = nc.dram_tensor("w8", [128, 2, 2, 128], FP8, kind="ExternalInput").ap()
    bp_d = nc.dram_tensor("bp", [128, 3], F32, kind="ExternalInput").ap()
    out_d = nc.dram_tensor("out", [B_PER, WIDTH, L], BF16,
                           kind="ExternalOutput").ap()

    # [b, (k p), l] -> [b, p, k, l]: a chunk slice is a [128, KC, ST] DMA
    # with contiguous rows
    xs_r = xs_d.rearrange("b (k p) l -> b p k l", p=128)
    x8_r = x8_d.rearrange("b (k p) l -> b p k l", p=128)

    with tile.TileContext(nc) as tc, ExitStack() as ctx:
        wpool = ctx.enter_context(tc.tile_pool(name="weights", bufs=1))
        io = ctx.enter_context(tc.tile_pool(name="io", bufs=5))
        obuf = ctx.enter_context(tc.tile_pool(name="obuf", bufs=2))
        acts = ctx.enter_context(tc.tile_pool(name="acts", bufs=2))
        ps_q = ctx.enter_context(tc.tile_pool(name="ps_q", bufs=2,
                                              space="PSUM"))
        ps_rz = ctx.enter_context(tc.tile_pool(name="ps_rz", bufs=1,
                                               space="PSUM"))

        # weights first (small, every matmul needs them), spread over the
        # three DMA-capable queues so the triggers don't serialize.
        w_sb = wpool.tile([128, 1920], BF16, tag="wp")
        nc.sync.dma_start(w_sb[:], wp_d[:])
        w8_sb = wpool.tile([128, 2, 2, 128], FP8, tag="w8")
        nc.gpsimd.dma_start(w8_sb[:], w8_d[:])
        b_sb = wpool.tile([128, 3], F32, tag="bp")
        nc.scalar.dma_start(b_sb[:], bp_d[:])

        # weight slices: per gate g (0=q,1=r,2=z): W chunks at
        # [:, g*512 + k*128 : +128], U at [:, 1536 + g*128 : +128]
        def wslice(g, k):
            return w_sb[:, g * 512 + k * 128: g * 512 + (k + 1) * 128]

        def uslice(g):
            return w_sb[:, 1536 + g * 128: 1536 + (g + 1) * 128]

        for b_i in range(B_PER):
            for l0 in range(0, L, ST):
                xs_t = io.tile([128, KC, ST], BF16, tag="xs_t")
                nc.sync.dma_start(xs_t[:], xs_r[b_i][:, :, l0:l0 + ST])
                x8_t = io.tile([128, KC, ST], FP8, tag="x8_t")
                nc.gpsimd.dma_start(x8_t[:], x8_r[b_i][:, :, l0:l0 + ST])
                z_t = io.tile([128, ST], BF16, tag="z_t")
                nc.sync.dma_start(z_t[:], zp_d[b_i][:, l0:l0 + ST])
                ob = obuf.tile([128, ST], BF16, tag="ob")

                q_ps = ps_q.tile([128, ST], F32, tag="q_ps")
                r_ps = ps_rz.tile([128, ST], F32, tag="r_ps")
                zt_ps = ps_rz.tile([128, ST], F32, tag="zt_ps")

                # ---- r gate first: its sigmoid + r*z run on ACT/DVE while
                # the q/zt matmuls stream, so rz is ready well before the
                # Uz matmuls at the end of this supertile's stream.
                for k2 in range(2):
                    for h in range(2):
                        nc.tensor.matmul(
                            r_ps[:, h * NT:(h + 1) * NT], w8_sb[:, k2],
                            x8_t[:, 2 * k2:2 * k2 + 2, h * NT:(h + 1) * NT],
                            start=(k2 == 0), stop=False,
                            perf_mode=mybir.MatmulPerfMode.DoubleRow)
                ur = uslice(1)
                for h in range(2):
                    nc.tensor.matmul(r_ps[:, h * NT:(h + 1) * NT], ur,
                                     z_t[:, h * NT:(h + 1) * NT],
                                     start=False, stop=True)
                r_s = acts.tile([128, ST], BF16, tag="r_s")
                nc.scalar.activation(r_s[:], r_ps[:],
                                     mybir.ActivationFunctionType.Sigmoid,
                                     bias=b_sb[:, 1:2], scale=1.0 / WRS)
                rz = acts.tile([128, ST], BF16, tag="rz")
                nc.vector.tensor_mul(rz[:], r_s[:], z_t[:])

                # ---- q gate
                for k in range(KC):
                    w = wslice(0, k)
                    for h in range(2):
                        nc.tensor.matmul(
                            q_ps[:, h * NT:(h + 1) * NT], w,
                            xs_t[:, k, h * NT:(h + 1) * NT],
                            start=(k == 0), stop=False)
                uq = uslice(0)
                for h in range(2):
                    nc.tensor.matmul(q_ps[:, h * NT:(h + 1) * NT], uq,
                                     z_t[:, h * NT:(h + 1) * NT],
                                     start=False, stop=True)
                q_s = acts.tile([128, ST], BF16, tag="q_s")
                nc.scalar.activation(q_s[:], q_ps[:],
                                     mybir.ActivationFunctionType.Sigmoid,
                                     bias=b_sb[:, 0:1])

                # ---- zt gate: W part, then Uz@(r*z) at the stream tail
                for k in range(KC):
                    w = wslice(2, k)
                    for h in range(2):
                        nc.tensor.matmul(
                            zt_ps[:, h * NT:(h + 1) * NT], w,
                            xs_t[:, k, h * NT:(h + 1) * NT],
                            start=(k == 0), stop=False)
                uz = uslice(2)
                for h in range(2):
                    nc.tensor.matmul(zt_ps[:, h * NT:(h + 1) * NT], uz,
                                     rz[:, h * NT:(h + 1) * NT],
                                     start=False, stop=True)
                zt_s = acts.tile([128, ST], BF16, tag="zt_s")
                nc.scalar.activation(zt_s[:], zt_ps[:],
                                     mybir.ActivationFunctionType.Tanh,
                                     bias=b_sb[:, 2:3])

                # ---- combine: out = zt + q*(z - zt), bf16 on DVE
                diff = acts.tile([128, ST], BF16, tag="diff")
                nc.vector.tensor_sub(diff[:], z_t[:], zt_s[:])
                prod = acts.tile([128, ST], BF16, tag="prod")
                nc.vector.tensor_mul(prod[:], q_s[:], diff[:])
                nc.vector.tensor_add(ob[:], zt_s[:], prod[:])
                nc.gpsimd.dma_start(out_d[b_i][:, l0:l0 + ST], ob[:])

    nc.compile()
    _module_cache[key] = nc
    return nc


def _pack_weights(inputs):
    # wp [128, 1920] bf16: per partition p:
    #   [g=q,r,z][k=0..3]: wp[p, g*512+k*128+o] = Wg_w[o, k*128+p]
    #   [g]: wp[p, 1536+g*128+o] = Ug_w[o, p]   (Ur pre-scaled by WRS)
    wp = np.empty((128, 1920), np.float32)
    bp = np.empty((128, 3), np.float32)
    for g, (wn, un, wbn, ubn) in enumerate((
        ("Wq_w", "Uq_w", "Wq_b", "Uq_b"),
        ("Wr_w", "Ur_w", "Wr_b", "Ur_b"),
        ("Wz_w", "Uz_w", "Wz_b", "Uz_b"),
    )):
        w = np.asarray(inputs[wn], np.float32)       # [128 out, 512 in]
        wp[:, g * 512:(g + 1) * 512] = (
            w.reshape(128, KC, 128).transpose(2, 1, 0).reshape(128, 512))
        us = WRS if g == 1 else 1.0
        wp[:, 1536 + g * 128: 1536 + (g + 1) * 128] = (
            us * np.asarray(inputs[un], np.float32).T)
        bp[:, g] = (np.asarray(inputs[wbn], np.float32)
                    + np.asarray(inputs[ubn], np.float32))
    # w8 [128, k2, j, o] fp8: WRS * Wr_w[o, (2*k2+j)*128 + p]
    wr = np.asarray(inputs["Wr_w"], np.float32)      # [128, 512]
    w8 = (WRS * wr.reshape(128, 2, 2, 128).transpose(3, 1, 2, 0))
    return (np.ascontiguousarray(wp.astype(ml_dtypes.bfloat16)),
            np.ascontiguousarray(w8.astype(ml_dtypes.float8_e4m3)),
            np.ascontiguousarray(bp))


def _run(inputs, trace=False, **run_kwargs):
    xs = np.asarray(inputs["xs"], dtype=np.float32)
    zp = np.asarray(inputs["z_prev"], dtype=np.float32)
    assert xs.shape == (B, IN_DIM, L) and zp.shape == (B, WIDTH, L)
    xs_bf = np.ascontiguousarray(xs.astype(ml_dtypes.bfloat16))
    xs_f8 = np.ascontiguousarray(xs.astype(ml_dtypes.float8_e4m3))
    zp_bf = np.ascontiguousarray(zp.astype(ml_dtypes.bfloat16))
    wp, w8, bp = _pack_weights(inputs)

    nc = _build()
    in_maps = []
    for c in range(N_CORES):
        m = {"xs": np.ascontiguousarray(xs_bf[c * B_PER:(c + 1) * B_PER]),
             "x8": np.ascontiguousarray(xs_f8[c * B_PER:(c + 1) * B_PER]),
             "zp": np.ascontiguousarray(zp_bf[c * B_PER:(c + 1) * B_PER]),
             "wp": wp, "w8": w8, "bp": bp}
        in_maps.append(m)

    res = run_bass_kernel_spmd(nc, in_maps, core_ids=list(range(N_CORES)),
                               trace=trace, **run_kwargs)
    out = np.concatenate(
        [np.asarray(res.results[c]["out"], dtype=np.float32)
         for c in range(N_CORES)], axis=0)
    return out, res


def kernel(**inputs):
    out, _ = _run(inputs, trace=False)
    return out
